# revision 7
# baseline (speedup 1.0000x reference)
import numpy as np

_CACHE = {}

N_CORES = 8
TOK = 16384
TOK_PER = TOK // N_CORES  # 2048 tokens per core
DIM = 2048
NE = 64
TOPK = 8
KC = 128            # contraction chunk (partition dim)
NK = DIM // KC      # 16 chunks
NT = 512            # token tile = one f32 PSUM bank
NJ = TOK_PER // NT  # 4 token tiles
WCOLS = NK * NE     # 1024 weight columns (all chunks, packed)

# Slab layout (chunk ids per dma_start). Slab 0 carries the packed
# weights + chunk 0 so the PE can start early; the last slab is a single
# chunk so little matmul work remains after the final byte lands.
SLAB_CHUNKS = [[0], [1, 2], [3, 4], [5, 6], [7, 8], [9, 10], [11, 12], [13, 14], [15]]


def _mm_schedule():
    # (chunk k, token tile j) -> PE column group g (0 => PSUM rows 0:64,
    # 1 => rows 64:128). The host sums the halves, so assignment is free;
    # pick it so the two groups stay concurrently busy in every slab.
    order = []  # emission order: (k, j, g)
    for si, ks in enumerate(SLAB_CHUNKS):
        if len(ks) == 2:
            a, b = ks
            for j in range(NJ):
                order.append((a, j, 0))
                order.append((b, j, 1))
        else:
            k = ks[0]
            for j in range(NJ):
                order.append((k, j, j % 2))
    # start/stop per (j, g) accumulation region
    first, last = {}, {}
    for idx, (k, j, g) in enumerate(order):
        if (j, g) not in first:
            first[(j, g)] = idx
        last[(j, g)] = idx
    return [
        (k, j, g, first[(j, g)] == idx, last[(j, g)] == idx)
        for idx, (k, j, g) in enumerate(order)
    ]


def _build():
    import concourse.bass as bass
    import concourse.tile as tile
    from concourse import bacc, mybir

    nc = bacc.Bacc(
        "TRN2",
        target_bir_lowering=False,
        debug=False,
        enable_asserts=False,
        num_devices=N_CORES,
    )
    # xpk[:, 0:WCOLS] = packed weights: wpk[p, k*NE + e] = W[e, k*KC + p]
    # xpk[:, WCOLS + k*TOK_PER + t] = x[t, k*KC + p]
    xpk = nc.dram_tensor(
        "xpk", (KC, WCOLS + NK * TOK_PER), mybir.dt.float16, kind="ExternalInput"
    ).ap()
    # packed output: rows 0:64 / 64:128 are the two partial-logit halves
    # (host sums them); columns = tokens of the core shard.
    out = nc.dram_tensor("opk", (KC, TOK_PER), mybir.dt.float16, kind="ExternalOutput").ap()

    with tile.TileContext(nc) as tc:
        with (
            tc.tile_pool(name="xpool", bufs=len(SLAB_CHUNKS)) as xpool,
            tc.tile_pool(name="opool", bufs=1) as opool,
            tc.tile_pool(name="psum", bufs=NJ, space=bass.MemorySpace.PSUM) as psum,
        ):
            chunk_rhs = {}  # k -> (tile, col offset of chunk k within tile)
            col = 0
            for si, ks in enumerate(SLAB_CHUNKS):
                w = WCOLS if si == 0 else 0
                xt = xpool.tile([KC, w + len(ks) * TOK_PER], mybir.dt.float16)
                nc.sync.dma_start(xt[:], xpk[:, col:col + w + len(ks) * TOK_PER])
                col += w + len(ks) * TOK_PER
                for ci, k in enumerate(ks):
                    chunk_rhs[k] = (xt, w + ci * TOK_PER)
                if si == 0:
                    wsb = xt  # weights live in slab 0's first WCOLS columns

            accs = [
                psum.tile([KC, NT], mybir.dt.float32, tag=f"acc{j}", bufs=1, name=f"acc{j}")
                for j in range(NJ)
            ]
            o128 = opool.tile([KC, TOK_PER], mybir.dt.float16)

            for (k, j, g, start, stop) in _mm_schedule():
                xt, off = chunk_rhs[k]
                nc.tensor.matmul(
                    accs[j][g * NE:(g + 1) * NE, :],
                    wsb[:, k * NE:(k + 1) * NE],
                    xt[:, off + j * NT:off + (j + 1) * NT],
                    start=start,
                    stop=stop,
                )
            for j in range(NJ):
                nc.vector.tensor_copy(o128[:, j * NT:(j + 1) * NT], accs[j][:])
            nc.scalar.dma_start(out[:, 0:2 * NT], o128[:, 0:2 * NT])
            nc.sync.dma_start(out[:, 2 * NT:4 * NT], o128[:, 2 * NT:4 * NT])
    nc.compile()
    return nc


def _make_in_maps(x, W):
    x = np.asarray(x, dtype=np.float32)
    W = np.asarray(W, dtype=np.float32)
    WT = W.T.astype(np.float16)  # [DIM, NE]
    wpk = WT.reshape(NK, KC, NE).transpose(1, 0, 2).reshape(KC, WCOLS)
    in_maps = []
    for i in range(N_CORES):
        xs = x[i * TOK_PER:(i + 1) * TOK_PER]
        xT = xs.T.astype(np.float16)  # [DIM, TOK_PER]
        xp = xT.reshape(NK, KC, TOK_PER).transpose(1, 0, 2).reshape(KC, NK * TOK_PER)
        in_maps.append({"xpk": np.ascontiguousarray(np.concatenate([wpk, xp], axis=1))})
    return in_maps


def kernel(x, W):
    from concourse import bass_utils

    if "nc" not in _CACHE:
        _CACHE["nc"] = _build()
    nc = _CACHE["nc"]

    in_maps = _make_in_maps(x, W)
    res = bass_utils.run_bass_kernel_spmd(nc, in_maps, list(range(N_CORES)))
    parts = []
    for r in res.results:
        o = np.asarray(r["opk"], dtype=np.float32)  # [128, TOK_PER]
        parts.append((o[:NE, :] + o[NE:, :]).T)     # [TOK_PER, NE]
    logits = np.concatenate(parts, axis=0)

    m = logits.max(axis=-1, keepdims=True)
    e = np.exp(logits - m)
    scores = e / e.sum(axis=-1, keepdims=True)
    idx = np.argsort(-scores, axis=-1, kind="stable")[:, :TOPK].astype(np.int32)
    w = np.take_along_axis(scores, idx, axis=-1).astype(np.float32)
    return w, idx


# revision 8
# speedup vs baseline: 1.2126x; 1.2126x over previous
import numpy as np

_CACHE = {}

N_CORES = 8
TOK = 16384
TOK_PER = TOK // N_CORES  # 2048 tokens per core
DIM = 2048
NE = 64
TOPK = 8
KC = 128            # contraction chunk (partition dim)
NK = DIM // KC      # 16 chunks
NT = 512            # token tile = one f32 PSUM bank
NJ = TOK_PER // NT  # 4 token tiles
WCOLS = NK * NE     # 1024 weight columns (all chunks, packed)
NSLAB = NK // 2     # 8 slabs of 2 chunks; slab 0 also carries the weights


def _build():
    import concourse.bass as bass
    import concourse.tile as tile  # noqa: F401  (import parity with env)
    from concourse import bacc, mybir

    nc = bacc.Bacc(
        "TRN2",
        target_bir_lowering=False,
        debug=False,
        enable_asserts=False,
        num_devices=N_CORES,
    )
    # xpk[:, 0:WCOLS] = packed weights: wpk[p, k*NE + e] = W[e, k*KC + p]
    # xpk[:, WCOLS + k*TOK_PER + t] = x[t, k*KC + p]
    xpk = nc.dram_tensor(
        "xpk", (KC, WCOLS + NK * TOK_PER), mybir.dt.float16, kind="ExternalInput"
    ).ap()
    # packed output: rows 0:64 / 64:128 are the two partial-logit halves
    # (host sums them); columns = tokens of the core shard.
    out = nc.dram_tensor("opk", (KC, TOK_PER), mybir.dt.float16, kind="ExternalOutput").ap()

    # ---- static allocations (no reuse; everything lives for the kernel) ----
    slabs = []
    for s in range(NSLAB):
        w = WCOLS if s == 0 else 0
        h = nc.alloc_sbuf_tensor(f"xs{s}", [KC, w + 2 * TOK_PER], mybir.dt.float16)
        slabs.append(h.ap())
    o128 = nc.alloc_sbuf_tensor("o128", [KC, TOK_PER], mybir.dt.float16).ap()
    accs = [nc.alloc_psum_tensor(f"acc{j}", [KC, NT], mybir.dt.float32).ap() for j in range(NJ)]

    s_in = [nc.alloc_semaphore(f"sin{s}") for s in range(NSLAB)]
    s_acc = nc.alloc_semaphore("sacc")
    s_cast = nc.alloc_semaphore("scast")
    s_out = nc.alloc_semaphore("sout")
    all_sems = s_in + [s_acc, s_cast, s_out]
    lo = min(s.num for s in all_sems)
    hi = max(s.num for s in all_sems)

    # The NEFF may be executed more than once per load: clear our sems
    # before any engine consumes them, then barrier.
    nc.gpsimd.sem_clear(range(lo, hi + 1))
    nc.all_engine_barrier()

    # ---- input DMAs (HWDGE, sync ring), issued up front ----
    col = 0
    for s in range(NSLAB):
        w = WCOLS if s == 0 else 0
        n = w + 2 * TOK_PER
        nc.sync.dma_start(slabs[s][:], xpk[:, col:col + n]).then_inc(s_in[s], 16)
        col += n
    wsb = slabs[0]

    # ---- matmuls: per slab, chunk pair (a -> PE col group 0 / PSUM rows
    # 0:64, b -> group 1 / rows 64:128), interleaved over token tiles ----
    for s in range(NSLAB):
        nc.tensor.wait_ge(s_in[s], 16)
        xt = slabs[s]
        ka, kb = 2 * s, 2 * s + 1
        xoff = WCOLS if s == 0 else 0
        for j in range(NJ):
            nc.tensor.matmul(
                accs[j][0:NE, :],
                wsb[:, ka * NE:(ka + 1) * NE],
                xt[:, xoff + j * NT:xoff + (j + 1) * NT],
                start=(ka == 0),
                stop=(ka == NK - 2),
            )
            mm_b = nc.tensor.matmul(
                accs[j][NE:2 * NE, :],
                wsb[:, kb * NE:(kb + 1) * NE],
                xt[:, xoff + TOK_PER + j * NT:xoff + TOK_PER + (j + 1) * NT],
                start=(kb == 1),
                stop=(kb == NK - 1),
            )
            if s == NSLAB - 1:
                mm_b.then_inc(s_acc, 1)

    # ---- PSUM -> SBUF casts (DVE), then two output DMAs on both rings ----
    for j in range(NJ):
        nc.vector.wait_ge(s_acc, j + 1)
        nc.vector.tensor_copy(o128[:, j * NT:(j + 1) * NT], accs[j][:]).then_inc(s_cast, 1)
    nc.scalar.wait_ge(s_cast, 2)
    nc.scalar.dma_start(out[:, 0:2 * NT], o128[:, 0:2 * NT]).then_inc(s_out, 16)
    nc.sync.wait_ge(s_cast, 4)
    nc.sync.dma_start(out[:, 2 * NT:4 * NT], o128[:, 2 * NT:4 * NT]).then_inc(s_out, 16)
    nc.sync.wait_ge(s_out, 32)
    nc.compile()
    return nc


def _make_in_maps(x, W):
    x = np.asarray(x, dtype=np.float32)
    W = np.asarray(W, dtype=np.float32)
    WT = W.T.astype(np.float16)  # [DIM, NE]
    wpk = WT.reshape(NK, KC, NE).transpose(1, 0, 2).reshape(KC, WCOLS)
    in_maps = []
    for i in range(N_CORES):
        xs = x[i * TOK_PER:(i + 1) * TOK_PER]
        xT = xs.T.astype(np.float16)  # [DIM, TOK_PER]
        xp = xT.reshape(NK, KC, TOK_PER).transpose(1, 0, 2).reshape(KC, NK * TOK_PER)
        in_maps.append({"xpk": np.ascontiguousarray(np.concatenate([wpk, xp], axis=1))})
    return in_maps


def kernel(x, W):
    from concourse import bass_utils

    if "nc" not in _CACHE:
        _CACHE["nc"] = _build()
    nc = _CACHE["nc"]

    in_maps = _make_in_maps(x, W)
    res = bass_utils.run_bass_kernel_spmd(nc, in_maps, list(range(N_CORES)))
    parts = []
    for r in res.results:
        o = np.asarray(r["opk"], dtype=np.float32)  # [128, TOK_PER]
        parts.append((o[:NE, :] + o[NE:, :]).T)     # [TOK_PER, NE]
    logits = np.concatenate(parts, axis=0)

    m = logits.max(axis=-1, keepdims=True)
    e = np.exp(logits - m)
    scores = e / e.sum(axis=-1, keepdims=True)
    idx = np.argsort(-scores, axis=-1, kind="stable")[:, :TOPK].astype(np.int32)
    w = np.take_along_axis(scores, idx, axis=-1).astype(np.float32)
    return w, idx


# revision 10
# speedup vs baseline: 1.2434x; 1.0253x over previous
import numpy as np

_CACHE = {}

N_CORES = 8
TOK = 16384
TOK_PER = TOK // N_CORES  # 2048 tokens per core
DIM = 2048
NE = 64
TOPK = 8
KC = 128            # contraction chunk (partition dim)
NK = DIM // KC      # 16 chunks
NT = 512            # token tile = one f32 PSUM bank
NJ = TOK_PER // NT  # 4 token tiles
WCOLS = NK * NE     # 1024 weight columns (all chunks, packed)
HK = NK // 2        # chunks per half-block DMA


def _build():
    import concourse.bass as bass
    from concourse import bacc, mybir

    nc = bacc.Bacc(
        "TRN2",
        target_bir_lowering=False,
        debug=False,
        enable_asserts=False,
        num_devices=N_CORES,
    )
    # j-major packed input:
    #   xpk[:, 0:WCOLS] = weights, wpk[p, k*NE + e] = W[e, k*KC + p]
    #   then for each token tile j, for each chunk k: [128, NT] block of
    #   x[t, :] with xblk[p, t'] = x[j*NT + t', k*KC + p]
    xpk = nc.dram_tensor(
        "xpk", (KC, WCOLS + NK * TOK_PER), mybir.dt.float16, kind="ExternalInput"
    ).ap()
    # packed output: rows 0:64 even-chunk half, 64:128 odd-chunk half
    out = nc.dram_tensor("opk", (KC, TOK_PER), mybir.dt.float16, kind="ExternalOutput").ap()

    # ---- static allocations ----
    halves = []  # 2 per j
    for j in range(NJ):
        for h in range(2):
            w = WCOLS if (j == 0 and h == 0) else 0
            t = nc.alloc_sbuf_tensor(f"xs{j}_{h}", [KC, w + HK * NT], mybir.dt.float16)
            halves.append(t.ap())
    o128 = nc.alloc_sbuf_tensor("o128", [KC, TOK_PER], mybir.dt.float16).ap()
    accs = [nc.alloc_psum_tensor(f"acc{j}", [KC, NT], mybir.dt.float32).ap() for j in range(NJ)]

    s_in = [nc.alloc_semaphore(f"sin{i}") for i in range(2 * NJ)]
    s_acc = nc.alloc_semaphore("sacc")
    s_cast = nc.alloc_semaphore("scast")
    s_out = nc.alloc_semaphore("sout")
    all_sems = s_in + [s_acc, s_cast, s_out]
    lo = min(s.num for s in all_sems)
    hi = max(s.num for s in all_sems)

    # NEFF may execute more than once per load: clear our sems first.
    nc.gpsimd.sem_clear(range(lo, hi + 1))
    nc.all_engine_barrier()

    # ---- input DMAs (HWDGE / sync ring), issued up front ----
    col = 0
    for i in range(2 * NJ):
        w = WCOLS if i == 0 else 0
        n = w + HK * NT
        nc.sync.dma_start(halves[i][:], xpk[:, col:col + n]).then_inc(s_in[i], 16)
        col += n
    wsb = halves[0]

    # ---- matmuls: per token tile j, 16 chunks; even chunk -> PE col
    # group 0 (PSUM rows 0:64), odd -> group 1 (rows 64:128). Consecutive
    # same-group matmuls use different weights, so LDWEIGHTS pipelines. ----
    for j in range(NJ):
        for h in range(2):
            i = 2 * j + h
            nc.tensor.wait_ge(s_in[i], 16)
            xt = halves[i]
            xoff = WCOLS if i == 0 else 0
            for kk in range(HK):
                k = h * HK + kk
                g = k % 2
                mm = nc.tensor.matmul(
                    accs[j][g * NE:(g + 1) * NE, :],
                    wsb[:, k * NE:(k + 1) * NE],
                    xt[:, xoff + kk * NT:xoff + (kk + 1) * NT],
                    start=(k < 2),
                    stop=(k >= NK - 2),
                )
            if h == 1:
                mm.then_inc(s_acc, 1)
        # PSUM -> SBUF cast for this token tile (DVE), then store it
        nc.vector.wait_ge(s_acc, j + 1)
        nc.vector.tensor_copy(o128[:, j * NT:(j + 1) * NT], accs[j][:]).then_inc(s_cast, 1)
        eng = nc.scalar if j % 2 == 0 else nc.sync
        eng.wait_ge(s_cast, j + 1)
        eng.dma_start(out[:, j * NT:(j + 1) * NT], o128[:, j * NT:(j + 1) * NT]).then_inc(s_out, 16)
    nc.sync.wait_ge(s_out, 16 * NJ)
    nc.compile()
    return nc


def _make_in_maps(x, W):
    x = np.asarray(x, dtype=np.float32)
    W = np.asarray(W, dtype=np.float32)
    WT = W.T.astype(np.float16)  # [DIM, NE]
    wpk = WT.reshape(NK, KC, NE).transpose(1, 0, 2).reshape(KC, WCOLS)
    in_maps = []
    for i in range(N_CORES):
        xs = x[i * TOK_PER:(i + 1) * TOK_PER]
        xT = xs.T.astype(np.float16)  # [DIM, TOK_PER]
        # [NK, KC, NJ, NT] -> [KC, NJ, NK, NT]
        xp = (
            xT.reshape(NK, KC, NJ, NT)
            .transpose(1, 2, 0, 3)
            .reshape(KC, NK * TOK_PER)
        )
        in_maps.append({"xpk": np.ascontiguousarray(np.concatenate([wpk, xp], axis=1))})
    return in_maps


def kernel(x, W):
    from concourse import bass_utils

    if "nc" not in _CACHE:
        _CACHE["nc"] = _build()
    nc = _CACHE["nc"]

    in_maps = _make_in_maps(x, W)
    res = bass_utils.run_bass_kernel_spmd(nc, in_maps, list(range(N_CORES)))
    parts = []
    for r in res.results:
        o = np.asarray(r["opk"], dtype=np.float32)  # [128, TOK_PER]
        parts.append((o[:NE, :] + o[NE:, :]).T)     # [TOK_PER, NE]
    logits = np.concatenate(parts, axis=0)

    m = logits.max(axis=-1, keepdims=True)
    e = np.exp(logits - m)
    scores = e / e.sum(axis=-1, keepdims=True)
    idx = np.argsort(-scores, axis=-1, kind="stable")[:, :TOPK].astype(np.int32)
    w = np.take_along_axis(scores, idx, axis=-1).astype(np.float32)
    return w, idx
